# revision 1
# baseline (speedup 1.0000x reference)
"""Trainium2 Bass kernel for nn_GATv2Layer4View (4-view GATv2 + inter-view MHA).

Sharding: 8 cores = 4 graphs x 2 destination-halves (500 dst nodes each,
all 4 views per core).  Host slices each core's edge list (per the
data-parallel-over-B sharding hint); all numerics run on device.

Math (validated exact vs the reference, see oracle.py): the segment softmax
collapses because scores[e] = s_src[src_e] + s_dst[dst_e] and the dst term
is segment-constant, so with w[n,h] = exp(s_src[n,h]) (s_src nonzero only
for global node ids < 1000, the reference's raw-id gather quirk):

  out[b,v,d,:] = sum_{e in-block, dst=d} w[src]*h[b,v,src] /
                 ( indeg(d) + sum_{e: dst=d, src<1000} (w[src,h]-1) )

Both sums are computed on device by dma_gather of table rows + one-hot
matmul accumulation in PSUM, then the per-node 4-view MHA runs on chip.
"""
import math

import numpy as np

import concourse.bacc as bacc
import concourse.bass as bass
import concourse.mybir as mybir
from concourse.masks import make_identity
from concourse.tile import TileContext
from concourse.bass_utils import run_bass_kernel_spmd

# ---------------------------------------------------------------- drain patch
# This container's walrus only accepts one sync-wait on the NO_STRUCT Drain
# encoding; carry each global-clock component on its own single-wait SP nop.
import concourse.tile as _tile_mod
from concourse.vector_clock import ScopedClock, VectorClock


def _patched_drain_and_barrier(self, tick_clock, wait_clock):
    gc = tick_clock.global_clock
    n = len(gc)
    for i in range(n):
        t = gc[i]
        if t > 0:
            v = VectorClock([0] * i + [t] + [0] * (n - 1 - i))
            nop = self.nc.sync.nop(nofuse=True)
            wait_clock.add_sem_waits(nop.ins, ScopedClock({None: v}))
    self.nc.sync.drain()
    self.nc.all_engine_barrier()
    assert self.sems is not None
    popped = self.nc._tile_sem_poison_stack.pop()
    assert popped is self._sem_poison
    self.nc.clear_and_free_semaphores(list(self.sems.allocated().values()))
    self.nc.all_engine_barrier()


_tile_mod.TileContext._drain_and_barrier = _patched_drain_and_barrier
# ----------------------------------------------------------------------------




F32 = mybir.dt.float32
F32R = mybir.dt.float32r
I16 = mybir.dt.int16

B, V, N, IN_F, HEADS, OUT_F = 4, 4, 1000, 64, 4, 32
D = HEADS * OUT_F          # 128
NTOT = B * N               # 4000
NH = 500                   # dst nodes per core (half graph)
NCH = 4                    # dst chunks per core
CH = 125                   # dst nodes per chunk
NT = 8                     # n tiles per graph (1000 = 7*128 + 104)
SQ = 1.0 / math.sqrt(32.0)
PAD_DST = -1.0e9


def _ptile(nt):
    return 128 if nt < 7 else 104


# ============================================================= host-side prep
def _wrap_idx(idx):
    """int16 [n*128] -> [16, n*8] in dma_gather's wrapped layout."""
    n = idx.shape[0]
    assert n % 128 == 0
    return idx.reshape(n // 16, 16).T.astype(np.int16)


def _prep_core(b, half, src, dst, indeg_full, tg, td):
    """Build one core's edge-stream arrays (padded to tg/td tiles/chunk)."""
    base = b * N + half * NH
    in_half = (dst >= base) & (dst < base + NH)
    dl = dst - base

    g_m = in_half & (src // N == b)
    d_m = in_half & (src < N)

    def chunked(mask, srcvals, ntiles):
        gi = np.zeros(NCH * ntiles * 128, np.int16)
        gd = np.full(NCH * ntiles * 128, PAD_DST, np.float32)
        for c in range(NCH):
            m = mask & (dl // CH == c)
            cnt = int(m.sum())
            assert cnt <= ntiles * 128, f"chunk overflow {cnt} > {ntiles*128}"
            o = c * ntiles * 128
            gi[o:o + cnt] = srcvals[m]
            gd[o:o + cnt] = (dl[m] % CH).astype(np.float32)
        # wrap per chunk so each dma_gather call sees its own layout
        gi_w = np.concatenate(
            [_wrap_idx(gi[c * ntiles * 128:(c + 1) * ntiles * 128])
             for c in range(NCH)], axis=1)
        gd_t = gd.reshape(NCH * ntiles, 128).T.copy()
        return np.tile(gi_w, (8, 1)).copy(), gd_t

    gidx, gdst = chunked(g_m, (src - b * N).astype(np.int16), tg)
    didx, ddst = chunked(d_m, src.astype(np.int16), td)

    indeg = np.zeros((128, NCH * HEADS), np.float32)
    for c in range(NCH):
        seg = indeg_full[base + c * CH: base + (c + 1) * CH]
        indeg[:CH, c * HEADS:(c + 1) * HEADS] = seg[:, None]
    return gidx, gdst, didx, ddst, indeg


def _host_prep(x, W, att, in_proj_w, in_proj_b, out_proj_w, out_proj_b,
               bias, edge_index):
    ei = np.asarray(edge_index)
    src = np.concatenate([ei[0], np.arange(NTOT)]).astype(np.int64)
    dst = np.concatenate([ei[1], np.arange(NTOT)]).astype(np.int64)
    indeg_full = np.bincount(dst, minlength=NTOT).astype(np.float32)

    # fixed tile counts = max over cores/chunks (SPMD: one program)
    tg = td = 1
    for b in range(B):
        for half in range(2):
            base = b * N + half * NH
            in_half = (dst >= base) & (dst < base + NH)
            dl = dst - base
            for c in range(NCH):
                cm = in_half & (dl // CH == c)
                gc = int((cm & (src // N == b)).sum())
                dc = int((cm & (src < N)).sum())
                tg = max(tg, (gc + 127) // 128)
                td = max(td, (dc + 127) // 128)

    x = np.ascontiguousarray(np.asarray(x, np.float32))
    common = {
        "WTT": np.ascontiguousarray(np.asarray(W, np.float32).T),       # [64,128]
        "ATTS": _att_bd(np.asarray(att, np.float32)),                   # [128,4]
        "WQT": np.ascontiguousarray(np.asarray(in_proj_w[:D], np.float32).T),
        "WKT": np.ascontiguousarray(np.asarray(in_proj_w[D:2 * D], np.float32).T),
        "WVT": np.ascontiguousarray(np.asarray(in_proj_w[2 * D:], np.float32).T),
        "WOT": np.ascontiguousarray(np.asarray(out_proj_w, np.float32).T),
        "INB": np.ascontiguousarray(
            np.asarray(in_proj_b, np.float32).reshape(3, D).T),          # [128,3]
        "OBB": np.ascontiguousarray(np.stack(
            [np.asarray(out_proj_b, np.float32),
             np.asarray(bias, np.float32)], axis=1)),                    # [128,2]
        "IOTA": np.broadcast_to(
            np.arange(CH, dtype=np.float32), (128, CH)).copy(),
        "ONESBD": _ones_bd(),                                            # [128,4]
        "BD128": _bd128(),                                               # [4,128]
        "X00T": np.ascontiguousarray(x[0, 0].T),                         # [64,1000]
    }
    in_maps = []
    for k in range(8):
        b, half = k // 2, k % 2
        gidx, gdst, didx, ddst, indeg = _prep_core(
            b, half, src, dst, indeg_full, tg, td)
        m = dict(common)
        m["XBT"] = np.ascontiguousarray(
            x[b].transpose(2, 0, 1).reshape(IN_F, V * N))  # [64, V*1000]
        m["GIDX"], m["GDST"] = gidx, gdst
        m["DIDX"], m["DDST"] = didx, ddst
        m["INDEG"] = indeg
        m["ISB0"] = np.full((128, 1), 1.0 if b == 0 else 0.0, np.float32)
        in_maps.append(m)
    return in_maps, tg, td


def _att_bd(att):
    out = np.zeros((128, HEADS), np.float32)
    for h in range(HEADS):
        out[h * 32:(h + 1) * 32, h] = att[0, h, :32]
    return out


def _ones_bd():
    out = np.zeros((128, HEADS), np.float32)
    for h in range(HEADS):
        out[h * 32:(h + 1) * 32, h] = 1.0
    return out


def _bd128():
    out = np.zeros((HEADS, 128), np.float32)
    for h in range(HEADS):
        out[h, h * 32:(h + 1) * 32] = 1.0
    return out


# ============================================================ device program
def build_program(tg, td, phases=5):
    nc = bacc.Bacc("TRN2", target_bir_lowering=False)

    def din(name, shape, dt=F32):
        return nc.dram_tensor(name, shape, dt, kind="ExternalInput")

    XBT = din("XBT", [IN_F, V * N])
    X00T = din("X00T", [IN_F, N])
    WTT = din("WTT", [64, 128])
    ATTS = din("ATTS", [128, 4])
    WQT = din("WQT", [128, 128], F32R)
    WKT = din("WKT", [128, 128], F32R)
    WVT = din("WVT", [128, 128], F32R)
    WOT = din("WOT", [128, 128], F32R)
    INB = din("INB", [128, 3])
    OBB = din("OBB", [128, 2])
    IOTA = din("IOTA", [128, CH])
    ONESBD = din("ONESBD", [128, 4], F32R)
    BD128 = din("BD128", [4, 128], F32R)
    GIDX = din("GIDX", [128, NCH * tg * 8], I16)
    GDST = din("GDST", [128, NCH * tg])
    DIDX = din("DIDX", [128, NCH * td * 8], I16)
    DDST = din("DDST", [128, NCH * td])
    INDEG = din("INDEG", [128, NCH * HEADS])
    ISB0 = din("ISB0", [128, 1])
    OUTC = nc.dram_tensor("OUTC", [V, NH, D], F32, kind="ExternalOutput")

    TT = mybir.ActivationFunctionType

    with TileContext(nc) as tc:
        with (
            tc.tile_pool(name="cons", bufs=1) as cons,
            tc.tile_pool(name="per", bufs=1) as per,
            tc.tile_pool(name="dram", bufs=1, space="DRAM") as dram,
        ):
            def cload(name, ap, shape, dt=F32, eng=None):
                t = cons.tile(shape, dt, tag=name)
                (eng or nc.sync).dma_start(out=t[:], in_=ap)
                return t

            # critical path first: x00t/wtt/atts feed P2; xbt feeds P1
            x00t = cload("x00t", X00T[:], [64, N])
            wtt = cload("wtt", WTT[:], [64, 128], eng=nc.scalar)
            atts = cload("atts", ATTS[:], [128, 4], eng=nc.scalar)
            xbt = cons.tile([64, V * N], F32, tag="xbt")
            for v in range(V):
                nc.sync.dma_start(out=xbt[:, v * N:(v + 1) * N],
                                  in_=XBT[:, v * N:(v + 1) * N])
            isb0 = cload("isb0", ISB0[:], [128, 1], eng=nc.scalar)
            ident = cons.tile([128, 128], F32)
            make_identity(nc, ident[:])
            iota = cload("iota", IOTA[:], [128, CH], eng=nc.scalar)
            didx = cload("didx", DIDX[:], [128, NCH * td * 8], I16)
            ddst = cload("ddst", DDST[:], [128, NCH * td])
            onesbd = cload("onesbd", ONESBD[:], [128, 4], F32R, eng=nc.scalar)
            bd128 = cload("bd128", BD128[:], [4, 128], F32R, eng=nc.scalar)
            wq = cload("wq", WQT[:], [128, 128], F32R)
            wk = cload("wk", WKT[:], [128, 128], F32R)
            wv = cload("wv", WVT[:], [128, 128], F32R)
            wo = cload("wo", WOT[:], [128, 128], F32R)
            inb = cload("inb", INB[:], [128, 3], eng=nc.scalar)
            obb = cload("obb", OBB[:], [128, 2], eng=nc.scalar)
            gidx = cload("gidx", GIDX[:], [128, NCH * tg * 8], I16)
            gdst = cload("gdst", GDST[:], [128, NCH * tg], eng=nc.scalar)
            indeg = cload("indeg", INDEG[:], [128, NCH * HEADS])

            GT = dram.tile([N, V * D], F32R)          # w-scaled features
            WMT = dram.tile([N, 64], F32R)            # wm1 rows (cols 0:4)

            qb = per.tile([128, 1], F32)
            nc.vector.tensor_scalar_mul(out=qb[:], in0=inb[:, 0:1], scalar1=SQ)
            ob2 = per.tile([128, 1], F32)
            nc.vector.tensor_add(out=ob2[:], in0=obb[:, 0:1], in1=obb[:, 1:2])

            h_sb = per.tile([128, NT, V * D], F32)   # h[v] tiles, n-major
            lrT = per.tile([128, N], F32)            # lrelu(h00)^T, o-major
            wsel = per.tile([128, NT, 4], F32)       # 1 + isb0*(w-1), n-major
            wm1nm = per.tile([128, NT, 4], F32)      # w-1, n-major

            # ---------------- P2 first: h00T -> lrT -> s -> w tables
            with (
                tc.tile_pool(name="p2s", bufs=4) as p2s,
                tc.tile_pool(name="p2p", bufs=2, space="PSUM") as p2p,
            ):
                wT = per.tile([4, N], F32)
                for hh in range(2):
                    sl = slice(hh * 500, hh * 500 + 500)
                    h0_ps = p2p.tile([128, 500], F32, space="PSUM", tag="h0")
                    nc.tensor.matmul(out=h0_ps[:], lhsT=wtt[:], rhs=x00t[:, sl],
                                     start=True, stop=True)
                    sc2 = p2s.tile([128, 500], F32, tag="sc2")
                    nc.scalar.activation(out=sc2[:], in_=h0_ps[:],
                                         func=TT.Copy, scale=0.2)
                    nc.vector.tensor_tensor(out=lrT[:, sl], in0=h0_ps[:],
                                            in1=sc2[:], op=mybir.AluOpType.max)
                    s_ps = p2p.tile([4, 500], F32, space="PSUM", tag="sps")
                    nc.tensor.matmul(out=s_ps[:], lhsT=atts[:], rhs=lrT[:, sl],
                                     start=True, stop=True)
                    nc.scalar.activation(out=wT[:, sl], in_=s_ps[:], func=TT.Exp)
                for nt in range(NT):
                    p = _ptile(nt)
                    wn_ps = p2p.tile([128, 4], F32, space="PSUM", tag="wn")
                    nc.tensor.transpose(
                        out=wn_ps[:p, :], in_=wT[:, nt * 128:nt * 128 + p],
                        identity=ident[0:4, 0:4])
                    nc.vector.tensor_scalar_add(out=wm1nm[:p, nt, :],
                                                in0=wn_ps[:p, :], scalar1=-1.0)
                    wm1s = p2s.tile([128, 4], F32, tag="wm1s")
                    nc.vector.tensor_tensor(
                        out=wm1s[:p, :], in0=wm1nm[:p, nt, :],
                        in1=isb0[:p, 0:1].to_broadcast([p, 4]),
                        op=mybir.AluOpType.mult)
                    nc.vector.tensor_scalar_add(out=wsel[:p, nt, :],
                                                in0=wm1s[:p, :], scalar1=1.0)

                # WMT table (den gather source) - available early
                wmt = per.tile([128, NT, 64], F32R)
                nc.gpsimd.memset(wmt[:].bitcast(mybir.dt.int32), 0)
                for nt in range(NT):
                    p = _ptile(nt)
                    nc.vector.tensor_copy(out=wmt[:p, nt, 0:4],
                                          in_=wm1nm[:p, nt, :])
                nc.sync.dma_start(
                    out=WMT[:896, :].rearrange("(t p) f -> p t f", p=128),
                    in_=wmt[:, 0:7, :])
                nc.sync.dma_start(out=WMT[896:N, :], in_=wmt[:104, 7, :])

            # ---------------- P1: h = x @ W.T for all views (lhsT slices xbt)
            with (
                tc.tile_pool(name="p1s", bufs=4) as p1s,
                tc.tile_pool(name="p1p", bufs=8, space="PSUM") as p1p,
            ):
                for v in range(V):
                    for nt in range(NT):
                        p = _ptile(nt)
                        hp = p1p.tile([128, 128], F32, space="PSUM", tag="hp")
                        nc.tensor.matmul(
                            out=hp[:p, :],
                            lhsT=xbt[:, v * N + nt * 128: v * N + nt * 128 + p],
                            rhs=wtt[:], start=True, stop=True)
                        nc.any.tensor_copy(out=h_sb[:p, nt, v * D:(v + 1) * D],
                                           in_=hp[:p, :])

            if phases >= 3:
                # ------------ P3: g = wsel*h -> GT
                with tc.tile_pool(name="p3s", bufs=1) as p3s:
                    g_all = p3s.tile([128, NT, V * D], F32R)
                    for nt in range(NT):
                        p = _ptile(nt)
                        wb = bass.AP(wsel.tensor, wsel[:p, nt, :].offset,
                                     [wsel[:p, nt, :].ap[0], [0, V], [1, 4],
                                      [0, 32]])
                        nc.vector.tensor_tensor(
                            out=g_all[:p, nt, :].rearrange(
                                "p (a b c) -> p a b c", a=V, b=4),
                            in0=h_sb[:p, nt, :].rearrange(
                                "p (a b c) -> p a b c", a=V, b=4),
                            in1=wb, op=mybir.AluOpType.mult)
                    nc.sync.dma_start(
                        out=GT[:512, :].rearrange("(t p) f -> p t f", p=128),
                        in_=g_all[:, 0:4, :])
                    nc.sync.dma_start(
                        out=GT[512:896, :].rearrange("(t p) f -> p t f", p=128),
                        in_=g_all[:, 4:7, :])
                    nc.scalar.dma_start(out=GT[896:N, :], in_=g_all[:104, 7, :])

            if phases >= 4:
                # ------------ P4: gather + one-hot accumulate per chunk
                gat = per.tile([128, NCH, V * D], F32)
                with (
                    tc.tile_pool(name="p4s", bufs=3) as p4s,
                    tc.tile_pool(name="p4o", bufs=10) as p4o,
                    tc.tile_pool(name="p4p", bufs=4, space="PSUM") as p4p,
                ):
                    recs4 = []
                    for c in range(NCH):
                        wrows = p4s.tile([128, td, 64], F32R, tag="wrows")
                        nc.gpsimd.dma_gather(
                            out_ap=wrows[:], in_ap=WMT[:],
                            idxs_ap=didx[:, c * td * 8:(c + 1) * td * 8],
                            num_idxs=td * 128, num_idxs_reg=td * 128,
                            elem_size=64)
                        den_ps = p4p.tile([CH, 4], F32, space="PSUM", tag="den")
                        for t in range(td):
                            oh = p4o.tile([128, CH], F32R, tag="oh")
                            nc.vector.tensor_tensor(
                                out=oh[:],
                                in0=ddst[:, c * td + t, None].to_broadcast(
                                    [128, CH]),
                                in1=iota[:], op=mybir.AluOpType.is_equal)
                            nc.tensor.matmul(out=den_ps[:], lhsT=oh[:],
                                             rhs=wrows[:, t, 0:4],
                                             start=(t == 0), stop=(t == td - 1))
                        den = p4s.tile([CH, 4], F32, tag=f"dens{c}")
                        nc.vector.tensor_add(
                            out=den[:], in0=den_ps[:],
                            in1=indeg[0:CH, c * 4:(c + 1) * 4])
                        rec = p4s.tile([CH, 4], F32, tag=f"rec{c}")
                        nc.vector.reciprocal(out=rec[:], in_=den[:])
                        recs4.append(rec)
                    th = (tg + 1) // 2      # first-half tiles per chunk
                    for c in range(NCH):
                        halves = []
                        for hf, (t0, t1) in enumerate(((0, th), (th, tg))):
                            nt_h = t1 - t0
                            if nt_h == 0:
                                continue
                            gr = p4s.tile([128, th, V * D], F32R,
                                          tag=f"grows{hf}")
                            nc.gpsimd.dma_gather(
                                out_ap=gr[:, 0:nt_h, :], in_ap=GT[:],
                                idxs_ap=gidx[:, (c * tg + t0) * 8:
                                             (c * tg + t1) * 8],
                                num_idxs=nt_h * 128, num_idxs_reg=nt_h * 128,
                                elem_size=V * D)
                            halves.append((t0, nt_h, gr))
                        rec = recs4[c]
                        num_ps = p4p.tile([CH, V * D], F32, space="PSUM",
                                          tag="num")
                        for t0, nt_h, gr in halves:
                            for t in range(nt_h):
                                oh = p4o.tile([128, CH], F32R, tag="oh")
                                nc.vector.tensor_tensor(
                                    out=oh[:],
                                    in0=gdst[:, c * tg + t0 + t,
                                             None].to_broadcast([128, CH]),
                                    in1=iota[:], op=mybir.AluOpType.is_equal)
                                nc.tensor.matmul(
                                    out=num_ps[:], lhsT=oh[:],
                                    rhs=gr[:, t, :],
                                    start=(t0 + t == 0),
                                    stop=(t0 + t == tg - 1))
                        for h in range(HEADS):
                            nc.scalar.activation(
                                out=gat[0:CH, c, :].rearrange(
                                    "p (a b c) -> p a b c", a=V, b=4
                                )[:, :, h, :],
                                in_=num_ps[:].rearrange(
                                    "p (a b c) -> p a b c", a=V, b=4
                                )[:, :, h, :],
                                func=TT.Identity, scale=rec[:, h:h + 1])

            if phases >= 5:
                # ------------ P5: MHA over views
                with tc.tile_pool(name="p5s", bufs=3) as p5s, tc.tile_pool(name="p5b", bufs=2) as p5b:
                    gatT = per.tile([128, V, NH], F32R)
                    qT = per.tile([128, V, NH], F32)
                    kT = per.tile([128, V, NH], F32)
                    vT = per.tile([128, V, NH], F32)
                    with tc.tile_pool(name="p5p1", bufs=4,
                                      space="PSUM") as p5p1:
                        for v in range(V):
                            for c in range(NCH):
                                gT_ps = p5p1.tile([128, CH], F32, space="PSUM",
                                                  tag="gT")
                                nc.tensor.transpose(
                                    out=gT_ps[:],
                                    in_=gat[0:CH, c, v * D:(v + 1) * D],
                                    identity=ident[0:CH, 0:CH])
                                nc.any.tensor_copy(
                                    out=gatT[:, v, c * CH:(c + 1) * CH],
                                    in_=gT_ps[:])
                        for v in range(V):
                            for (wmat, dstt, biasap, scale) in (
                                (wq, qT, qb[:], SQ), (wk, kT, inb[:, 1:2], 1.0),
                                (wv, vT, inb[:, 2:3], 1.0),
                            ):
                                pp = p5p1.tile([128, NH], F32, space="PSUM",
                                               tag="qkv")
                                nc.tensor.matmul(out=pp[:], lhsT=wmat[:],
                                                 rhs=gatT[:, v, :],
                                                 start=True, stop=True)
                                nc.scalar.activation(out=dstt[:, v, :],
                                                     in_=pp[:],
                                                     func=TT.Identity,
                                                     bias=biasap, scale=scale)

                    ao = per.tile([128, V, NH], F32R)
                    with tc.tile_pool(name="p5p2", bufs=4,
                                      space="PSUM") as p5p2:
                        for vq in range(V):
                            e_all = p5b.tile([4, V * NH], F32, tag="eall")
                            for vk in range(V):
                                prod = p5s.tile([128, NH], F32R, tag="prod")
                                nc.vector.tensor_tensor(
                                    out=prod[:], in0=qT[:, vq, :],
                                    in1=kT[:, vk, :], op=mybir.AluOpType.mult)
                                sc_ps = p5p2.tile([4, NH], F32, space="PSUM",
                                                  tag="sc")
                                nc.tensor.matmul(out=sc_ps[:], lhsT=onesbd[:],
                                                 rhs=prod[:], start=True,
                                                 stop=True)
                                nc.scalar.activation(
                                    out=e_all[:, vk * NH:(vk + 1) * NH],
                                    in_=sc_ps[:], func=TT.Exp)
                            ssum = p5s.tile([4, NH], F32, tag="ssum")
                            nc.gpsimd.tensor_add(out=ssum[:], in0=e_all[:, 0:NH],
                                                 in1=e_all[:, NH:2 * NH])
                            ssum2 = p5s.tile([4, NH], F32, tag="ssum2")
                            nc.gpsimd.tensor_add(out=ssum2[:],
                                                 in0=e_all[:, 2 * NH:3 * NH],
                                                 in1=e_all[:, 3 * NH:4 * NH])
                            nc.gpsimd.tensor_add(out=ssum[:], in0=ssum[:],
                                                 in1=ssum2[:])
                            recs = p5s.tile([4, NH], F32, tag="recs")
                            nc.vector.reciprocal(out=recs[:], in_=ssum[:])
                            aw = p5b.tile([4, V * NH], F32R, tag="aw")
                            for vk in range(V):
                                aeng = nc.gpsimd if vk % 2 else nc.vector
                                aeng.tensor_tensor(
                                    out=aw[:, vk * NH:(vk + 1) * NH],
                                    in0=e_all[:, vk * NH:(vk + 1) * NH],
                                    in1=recs[:], op=mybir.AluOpType.mult)
                            tmps = []
                            for vk in range(V):
                                ab_ps = p5p2.tile([128, NH], F32, space="PSUM",
                                                  tag="ab")
                                nc.tensor.matmul(
                                    out=ab_ps[:], lhsT=bd128[:],
                                    rhs=aw[:, vk * NH:(vk + 1) * NH],
                                    start=True, stop=True)
                                tmp = p5s.tile([128, NH], F32, tag=f"tmp{vk}")
                                nc.vector.tensor_tensor(
                                    out=tmp[:], in0=vT[:, vk, :], in1=ab_ps[:],
                                    op=mybir.AluOpType.mult)
                                tmps.append(tmp)
                            t01 = p5s.tile([128, NH], F32, tag="t01")
                            nc.gpsimd.tensor_add(out=t01[:], in0=tmps[0][:],
                                                 in1=tmps[1][:])
                            t23 = p5s.tile([128, NH], F32, tag="t23")
                            nc.vector.tensor_add(out=t23[:], in0=tmps[2][:],
                                                 in1=tmps[3][:])
                            nc.gpsimd.tensor_add(out=ao[:, vq, :], in0=t01[:],
                                                 in1=t23[:])

                    with tc.tile_pool(name="p5p3", bufs=4,
                                      space="PSUM") as p5p3:
                        for vq in range(V):
                            mx_ps = p5p3.tile([128, NH], F32, space="PSUM",
                                              tag="mx")
                            nc.tensor.matmul(out=mx_ps[:], lhsT=wo[:],
                                             rhs=ao[:, vq, :], start=True,
                                             stop=True)
                            mx = p5s.tile([128, NH], F32, tag="mxs")
                            nc.scalar.activation(out=mx[:], in_=mx_ps[:],
                                                 func=TT.Identity, bias=ob2[:])
                            osb = p5s.tile([CH, NCH, 128], F32, tag="osb")
                            for c in range(NCH):
                                oT_ps = p5p3.tile([CH, 128], F32, space="PSUM",
                                                  tag="oT")
                                nc.tensor.transpose(
                                    out=oT_ps[:],
                                    in_=mx[:, c * CH:(c + 1) * CH],
                                    identity=ident[:])
                                nc.any.tensor_copy(out=osb[:, c, :],
                                                   in_=oT_ps[:])
                            nc.sync.dma_start(
                                out=OUTC[vq].rearrange("(c p) f -> p c f",
                                                       p=CH),
                                in_=osb[:])

    nc.compile()
    return nc


# ================================================================== kernel()
_CACHE = {}


def kernel(**inputs):
    in_maps, tg, td = _host_prep(**inputs)
    key = (tg, td)
    if key not in _CACHE:
        _CACHE[key] = build_program(tg, td)
    nc = _CACHE[key]
    res = run_bass_kernel_spmd(nc, in_maps, core_ids=list(range(8)))
    out = np.zeros((B, V, N, D), np.float32)
    for k in range(8):
        b, half = k // 2, k % 2
        out[b, :, half * NH:(half + 1) * NH, :] = res.results[k]["OUTC"]
    return out



# revision 15
# speedup vs baseline: 1.8425x; 1.8425x over previous
"""Trainium2 Bass kernel for nn_GATv2Layer4View (4-view GATv2 + inter-view MHA).

Sharding: 8 cores = 4 graphs x 2 destination-halves (500 dst nodes each, all
4 views per core).  Host builds per-core dense multiplicity matrices from
edge_index (index prep only); all numerics run on device.

Math (validated vs the reference): the segment softmax collapses because
scores[e] = s_src[src_e] + s_dst[dst_e] and the dst term is segment-constant,
so with w[n,h] = exp(s_src[n,h]) (s_src nonzero only for global ids < 1000,
the reference's raw-id gather quirk):

  out[b,v,d,:] = sum_s C[d,s] * wsel[s,h] * h[b,v,s] /
                 ( indeg(d) + sum_s0 D[d,s0] (w0[s0,h]-1) )

where C[d,s] = multiplicity of in-block edge (s->d) (self-loops included),
D[d,s0] = multiplicity of edges from global src < 1000, and wsel = w0 on the
b=0 cores and 1 elsewhere.  Both contractions run as dense bf16 matmuls
against host-uploaded C^T / D^T; the numerator is produced directly in
[feature, dst] layout (g^T @ C^T) so no transposes are needed anywhere.
The per-node 4-view MHA runs on-chip with PSUM-accumulated aggregation.
"""
import math

import numpy as np
import ml_dtypes

import concourse.bacc as bacc
import concourse.bass as bass
import concourse.mybir as mybir
from concourse.tile import TileContext
from concourse.bass_utils import run_bass_kernel_spmd

# ---------------------------------------------------------------- drain patch
# This container's walrus only accepts one sync-wait on the NO_STRUCT Drain
# encoding; carry each global-clock component on its own single-wait SP nop.
import concourse.tile as _tile_mod
from concourse.vector_clock import ScopedClock, VectorClock


def _patched_drain_and_barrier(self, tick_clock, wait_clock):
    gc = tick_clock.global_clock
    n = len(gc)
    for i in range(n):
        t = gc[i]
        if t > 0:
            v = VectorClock([0] * i + [t] + [0] * (n - 1 - i))
            nop = self.nc.sync.nop(nofuse=True)
            wait_clock.add_sem_waits(nop.ins, ScopedClock({None: v}))
    self.nc.sync.drain()
    self.nc.all_engine_barrier()
    assert self.sems is not None
    popped = self.nc._tile_sem_poison_stack.pop()
    assert popped is self._sem_poison
    self.nc.clear_and_free_semaphores(list(self.sems.allocated().values()))
    self.nc.all_engine_barrier()


_tile_mod.TileContext._drain_and_barrier = _patched_drain_and_barrier
# ----------------------------------------------------------------------------

F32 = mybir.dt.float32
BF16 = mybir.dt.bfloat16
BF = ml_dtypes.bfloat16

B, V, N, IN_F, HEADS, OUT_F = 4, 4, 1000, 64, 4, 32
D = HEADS * OUT_F          # 128
NTOT = B * N               # 4000
NH = 500                   # dst nodes per core (half graph)
NKT = 8                    # src tiles (1000 = 7*128 + 104)
SQ = 1.0 / math.sqrt(32.0)

# CONSTB (bf16) column layout
CB_WT = 0          # [64 rows, 128]  W^T
CB_ATT = 128       # [128, 4]        att_src block-diag
CB_WQ = 132        # [128, 128]      in_proj_w[:D]^T
CB_WK = 260
CB_WV = 388
CB_WO = 516        # out_proj_w^T
CB_ONE = 644       # [128, 32]       per-head ones (j -> head j%4)
CB_BD = 676        # [128, 128]      head->(h,d) expansion, vk-block-replicated
CB_SEL4 = 804      # [128, 4]        (vk-block,h)->h reduction
CB_REP16 = 808     # [4 rows, 128]   h->(vk-block,h) replication
CB_N = 936

# CONSTF (f32) column layout
CF_QB, CF_KB, CF_VB, CF_OB, CF_ISB0, CF_ISB0C = 0, 1, 2, 3, 4, 5
CF_INDEG = 6       # [4 rows, 500]
CF_N = 506


def _pt(kt):
    return 128 if kt < 7 else 104


# ============================================================= host-side prep
def _mult_matrix(dl, sl, rows, cols):
    """Dense multiplicity matrix [rows, cols] from index pairs."""
    idx = dl.astype(np.int64) * cols + sl.astype(np.int64)
    return np.bincount(idx, minlength=rows * cols).reshape(rows, cols)


def _kt_pack(mat_t):
    """[1000, 500] -> [128, 8*500] (src padded to 1024, kt-major)."""
    p = np.zeros((1024, NH), np.float32)
    p[:N] = mat_t
    return np.ascontiguousarray(
        p.reshape(NKT, 128, NH).transpose(1, 0, 2).reshape(128, NKT * NH)
    ).astype(BF)


def _host_prep(x, W, att, in_proj_w, in_proj_b, out_proj_w, out_proj_b,
               bias, edge_index):
    x = np.asarray(x, np.float32)
    ei = np.asarray(edge_index)
    src = np.concatenate([ei[0], np.arange(NTOT)]).astype(np.int64)
    dst = np.concatenate([ei[1], np.arange(NTOT)]).astype(np.int64)
    indeg_full = np.bincount(dst, minlength=NTOT).astype(np.float32)

    constb = np.zeros((128, CB_N), np.float32)
    constb[:IN_F, CB_WT:CB_WT + 128] = np.asarray(W, np.float32).T
    attf = np.asarray(att, np.float32)
    for h in range(HEADS):
        constb[h * 32:(h + 1) * 32, CB_ATT + h] = attf[0, h, :32]
        for j in range(32):
            if j % 4 == h:
                constb[h * 32:(h + 1) * 32, CB_ONE + j] = 1.0
        for vk in range(V):
            constb[vk * 32 + h, CB_BD + h * 32:CB_BD + (h + 1) * 32] = 1.0
            constb[vk * 32 + h, CB_SEL4 + h] = 1.0
            constb[h, CB_REP16 + vk * 32 + h] = 1.0
    ipw = np.asarray(in_proj_w, np.float32)
    constb[:, CB_WQ:CB_WQ + 128] = ipw[:D].T
    constb[:, CB_WK:CB_WK + 128] = ipw[D:2 * D].T
    constb[:, CB_WV:CB_WV + 128] = ipw[2 * D:].T
    constb[:, CB_WO:CB_WO + 128] = np.asarray(out_proj_w, np.float32).T
    constb = constb.astype(BF)

    ipb = np.asarray(in_proj_b, np.float32)
    x00t = np.ascontiguousarray(x[0, 0].T).astype(BF)

    in_maps = []
    for k in range(8):
        b, half = k // 2, k % 2
        base = b * N + half * NH
        m = (dst >= base) & (dst < base + NH)
        dl = dst[m] - base
        sl = src[m]
        in_blk = sl // N == b
        C = _mult_matrix(dl[in_blk], sl[in_blk] - b * N, NH, N)
        g0 = sl < N
        Dm = _mult_matrix(dl[g0], sl[g0], NH, N)

        constf = np.zeros((128, CF_N), np.float32)
        constf[:, CF_QB] = ipb[:D] * SQ
        constf[:, CF_KB] = ipb[D:2 * D]
        constf[:, CF_VB] = ipb[2 * D:]
        constf[:, CF_OB] = (np.asarray(out_proj_b, np.float32)
                            + np.asarray(bias, np.float32))
        constf[:, CF_ISB0] = 1.0 if b == 0 else 0.0
        constf[:, CF_ISB0C] = 0.0 if b == 0 else 1.0
        constf[:4, CF_INDEG:] = indeg_full[base:base + NH][None, :]

        in_maps.append({
            "X00T": x00t,
            "XT": np.ascontiguousarray(
                x[b].transpose(2, 0, 1).reshape(IN_F, V * N)).astype(BF),
            "CT": _kt_pack(C.T),
            "DT": _kt_pack(Dm.T),
            "CONSTB": constb,
            "CONSTF": np.ascontiguousarray(constf),
        })
    return in_maps


# ============================================================ device program
def build_program():
    nc = bacc.Bacc("TRN2", target_bir_lowering=False)

    X00T = nc.dram_tensor("X00T", [IN_F, N], BF16, kind="ExternalInput")
    XT = nc.dram_tensor("XT", [IN_F, V * N], BF16, kind="ExternalInput")
    CTD = nc.dram_tensor("CT", [128, NKT * NH], BF16, kind="ExternalInput")
    DTD = nc.dram_tensor("DT", [128, NKT * NH], BF16, kind="ExternalInput")
    CONSTB = nc.dram_tensor("CONSTB", [128, CB_N], BF16, kind="ExternalInput")
    CONSTF = nc.dram_tensor("CONSTF", [128, CF_N], F32, kind="ExternalInput")
    OUTC = nc.dram_tensor("OUTC", [V, D, NH], F32, kind="ExternalOutput")

    TT = mybir.ActivationFunctionType
    MUL = mybir.AluOpType.mult

    with TileContext(nc) as tc:
        with (
            tc.tile_pool(name="cons", bufs=1) as cons,
            tc.tile_pool(name="per", bufs=1) as per,
        ):
            # uploads: critical-path first
            x00t = cons.tile([IN_F, N], BF16, tag="x00t")
            nc.sync.dma_start(out=x00t[:], in_=X00T[:])
            cb = cons.tile([128, CB_N], BF16, tag="cb")
            nc.scalar.dma_start(out=cb[:], in_=CONSTB[:])
            cf = cons.tile([128, CF_N], F32, tag="cf")
            nc.scalar.dma_start(out=cf[:], in_=CONSTF[:])
            xt = cons.tile([IN_F, V * N], BF16, tag="xt")
            nc.sync.dma_start(out=xt[:], in_=XT[:])
            ct = cons.tile([128, NKT * NH], BF16, tag="ct")
            nc.sync.dma_start(out=ct[:], in_=CTD[:])
            dt = cons.tile([128, NKT * NH], BF16, tag="dt")
            nc.sync.dma_start(out=dt[:], in_=DTD[:])

            wt = cb[0:IN_F, CB_WT:CB_WT + 128]

            lrT = per.tile([128, N], BF16, tag="lrT")
            w_sb = per.tile([128, NKT, HEADS], F32, tag="w_sb")
            wsel = per.tile([128, NKT, HEADS], F32, tag="wsel")
            wm1b = per.tile([128, NKT, HEADS], BF16, tag="wm1b")
            g_sb = per.tile([128, NKT, V * D], BF16, tag="g_sb")
            gatT = per.tile([128, V, NH], BF16, tag="gatT")
            qT = per.tile([128, V, NH], BF16, tag="qT")
            kT = per.tile([128, V, NH], BF16, tag="kT")
            vT = per.tile([128, V, NH], BF16, tag="vT")

            # garbage rows (kt=7, p>104) must still hold benign values
            nc.gpsimd.memset(w_sb[:].bitcast(mybir.dt.int32), 0x3F800000)

            # ---------------- P2: w tables from x[0,0]
            with (
                tc.tile_pool(name="p2s", bufs=2) as p2s,
                tc.tile_pool(name="p2p", bufs=2, space="PSUM") as p2p,
                tc.tile_pool(name="p2q", bufs=4, space="PSUM") as p2q,
            ):
                for hh in range(2):
                    sl = slice(hh * NH, (hh + 1) * NH)
                    h0 = p2p.tile([128, NH], F32, space="PSUM", tag="h0")
                    nc.tensor.matmul(out=h0[:], lhsT=wt, rhs=x00t[:, sl],
                                     start=True, stop=True)
                    sc2 = p2s.tile([128, NH], BF16, tag="sc2")
                    nc.scalar.activation(out=sc2[:], in_=h0[:],
                                         func=TT.Copy, scale=0.2)
                    nc.vector.tensor_tensor(out=lrT[:, sl], in0=h0[:],
                                            in1=sc2[:],
                                            op=mybir.AluOpType.max)
                for nt in range(NKT):
                    p = _pt(nt)
                    sc = p2q.tile([128, HEADS], F32, space="PSUM", tag="sc")
                    nc.tensor.matmul(out=sc[:p, :],
                                     lhsT=lrT[:, nt * 128:nt * 128 + p],
                                     rhs=cb[:, CB_ATT:CB_ATT + 4],
                                     start=True, stop=True)
                    nc.scalar.activation(out=w_sb[:p, nt, :], in_=sc[:p, :],
                                         func=TT.Exp)
                wv_ = w_sb[:].rearrange("p a b -> p (a b)")
                nc.vector.tensor_scalar_add(
                    out=wm1b[:].rearrange("p a b -> p (a b)"), in0=wv_,
                    scalar1=-1.0)
                # wsel = w*isb0 + (1-isb0)
                isb0c_b = bass.AP(cf.tensor, cf[:, CF_ISB0C:CF_ISB0C + 1].offset,
                                  [cf[:, 0:1].ap[0], [0, NKT * HEADS]])
                nc.vector.scalar_tensor_tensor(
                    out=wsel[:].rearrange("p a b -> p (a b)"), in0=wv_,
                    scalar=cf[:, CF_ISB0:CF_ISB0 + 1], in1=isb0c_b,
                    op0=MUL, op1=mybir.AluOpType.add)

            # ---------------- P1+P3: h = x@W^T, g = wsel*h  (per src tile)
            with (
                tc.tile_pool(name="p1p", bufs=3, space="PSUM") as p1p,
            ):
                for kt in range(NKT):
                    p = _pt(kt)
                    hps = p1p.tile([128, V * D], F32, space="PSUM", tag="hps")
                    for v in range(V):
                        nc.tensor.matmul(
                            out=hps[:p, v * D:(v + 1) * D],
                            lhsT=xt[:, v * N + kt * 128:v * N + kt * 128 + p],
                            rhs=wt, start=True, stop=True)
                    wb = bass.AP(wsel.tensor, wsel[:p, kt, :].offset,
                                 [wsel[:p, kt, :].ap[0], [0, V], [1, HEADS],
                                  [0, OUT_F]])
                    nc.vector.tensor_tensor(
                        out=g_sb[:p, kt, :].rearrange(
                            "p (a b c) -> p a b c", a=V, b=HEADS),
                        in0=hps[:p, :].rearrange(
                            "p (a b c) -> p a b c", a=V, b=HEADS),
                        in1=wb, op=MUL)

            # ---------------- P4: numT_v = g_v^T @ C^T, denT = wm1^T @ D^T
            with (
                tc.tile_pool(name="p4p", bufs=1, space="PSUM") as p4p,
                tc.tile_pool(name="p4q", bufs=2, space="PSUM") as p4q,
                tc.tile_pool(name="p4s", bufs=2) as p4s,
            ):
                nums = p4p.tile([128, V, 512], F32, space="PSUM", tag="nums")
                dens = p4q.tile([HEADS, NH], F32, space="PSUM", tag="dens")
                for kt in range(NKT):
                    p = _pt(kt)
                    st, sp_ = kt == 0, kt == NKT - 1
                    for v in range(V):
                        nc.tensor.matmul(
                            out=nums[:, v, 0:NH],
                            lhsT=g_sb[:p, kt, v * D:(v + 1) * D],
                            rhs=ct[:p, kt * NH:(kt + 1) * NH],
                            start=st, stop=sp_)
                    nc.tensor.matmul(
                        out=dens[:], lhsT=wm1b[:p, kt, :],
                        rhs=dt[:p, kt * NH:(kt + 1) * NH],
                        start=st, stop=sp_)
                denf = p4s.tile([HEADS, NH], F32, tag="denf")
                nc.vector.tensor_add(out=denf[:], in0=dens[:],
                                     in1=cf[0:4, CF_INDEG:])
                rec4g = p4s.tile([HEADS, NH], BF16, tag="rec4g")
                with nc.allow_low_precision(reason="bf16 alpha ok"):
                    nc.vector.reciprocal(out=rec4g[:], in_=denf[:])
                recx = p4q.tile([128, NH], F32, space="PSUM", tag="recx")
                nc.tensor.matmul(out=recx[:], lhsT=cb[0:4, CB_BD:CB_BD + 128],
                                 rhs=rec4g[:], start=True, stop=True)
                recxb = p4s.tile([128, NH], BF16, tag="recxb")
                nc.scalar.activation(out=recxb[:], in_=recx[:], func=TT.Copy)
                for v in range(V):
                    nc.vector.tensor_tensor(out=gatT[:, v, :],
                                            in0=nums[:, v, 0:NH],
                                            in1=recxb[:], op=MUL)

            # ---------------- P5a: q/k/v projections
            with tc.tile_pool(name="p5p", bufs=4, space="PSUM") as p5p:
                for v in range(V):
                    for (col, dstt, bcol, scale, eng) in (
                        (CB_WQ, qT, CF_QB, SQ, "q"),
                        (CB_WK, kT, CF_KB, 1.0, "k"),
                        (CB_WV, vT, CF_VB, 1.0, "v"),
                    ):
                        pp = p5p.tile([128, NH], F32, space="PSUM", tag="qkv")
                        nc.tensor.matmul(out=pp[:],
                                         lhsT=cb[:, col:col + 128],
                                         rhs=gatT[:, v, :],
                                         start=True, stop=True)
                        if eng == "k":
                            nc.vector.tensor_scalar_add(
                                out=dstt[:, v, :], in0=pp[:],
                                scalar1=cf[:, bcol:bcol + 1])
                        else:
                            nc.scalar.activation(
                                out=dstt[:, v, :], in_=pp[:], func=TT.Identity,
                                bias=cf[:, bcol:bcol + 1], scale=scale)

            # ---------------- P5b: attention over views + out proj
            with (
                tc.tile_pool(name="p5s", bufs=2) as p5s,
                tc.tile_pool(name="p5e", bufs=2) as p5e,
                tc.tile_pool(name="a16", bufs=1, space="PSUM") as a16,
                tc.tile_pool(name="apb", bufs=1, space="PSUM") as apb,
                tc.tile_pool(name="mxp", bufs=2, space="PSUM") as mxp,
            ):
                for vq in range(V):
                    prod = p5s.tile([128, V, NH], BF16, tag="prod")
                    qv = qT[:, vq, :]
                    qb_ap = bass.AP(qT.tensor, qv.offset,
                                    [qv.ap[0], [0, V], [1, NH]])
                    nc.vector.tensor_tensor(out=prod[:], in0=qb_ap,
                                            in1=kT[:], op=MUL)
                    # vk blocks at partition 0/32 of two half-tiles (PE can
                    # only address partition bases 0/32/64)
                    s2 = a16.tile([64, 2, 512], F32, space="PSUM", tag="s2")
                    for vk in range(V):
                        nc.tensor.matmul(
                            out=s2[(vk % 2) * 32:(vk % 2) * 32 + 32, vk // 2, 0:NH],
                            lhsT=cb[:, CB_ONE:CB_ONE + 32],
                            rhs=prod[:, vk, :], start=True, stop=True)
                    e2 = p5e.tile([64, 2, NH], BF16, tag="e2")
                    nc.scalar.activation(out=e2[:], in_=s2[:, :, 0:NH], func=TT.Exp)
                    ssum = a16.tile([HEADS, NH], F32, space="PSUM", tag="ssum")
                    for half in range(2):
                        nc.tensor.matmul(out=ssum[:],
                                         lhsT=cb[0:64, CB_SEL4:CB_SEL4 + 4],
                                         rhs=e2[:, half, :],
                                         start=(half == 0), stop=(half == 1))
                    rec4 = p5s.tile([HEADS, NH], BF16, tag="rec4")
                    with nc.allow_low_precision(reason="bf16 softmax recip"):
                        nc.vector.reciprocal(out=rec4[:], in_=ssum[:])
                    rec64 = a16.tile([64, NH], F32, space="PSUM", tag="r64")
                    nc.tensor.matmul(out=rec64[:],
                                     lhsT=cb[0:4, CB_REP16:CB_REP16 + 64],
                                     rhs=rec4[:], start=True, stop=True)
                    rec64b = p5e.tile([64, NH], BF16, tag="r64b")
                    nc.scalar.activation(out=rec64b[:], in_=rec64[:],
                                         func=TT.Copy)
                    en2 = p5e.tile([64, 2, NH], BF16, tag="en2")
                    r64v = rec64b[:]
                    r64_ap = bass.AP(rec64b.tensor, r64v.offset,
                                     [r64v.ap[0], [0, 2], [1, NH]])
                    nc.vector.tensor_tensor(out=en2[:], in0=e2[:],
                                            in1=r64_ap, op=MUL)
                    absb = p5s.tile([128, V, NH], BF16, tag="absb")
                    for pair in range(2):
                        abp = apb.tile([128, 2, 512], F32, space="PSUM",
                                       tag="abp")
                        for j in range(2):
                            vk = pair * 2 + j
                            blk = (vk % 2) * 32
                            nc.tensor.matmul(
                                out=abp[:, j, 0:NH],
                                lhsT=cb[blk:blk + 4, CB_BD:CB_BD + 128],
                                rhs=en2[blk:blk + 4, vk // 2, :],
                                start=True, stop=True)
                        nc.scalar.activation(
                            out=absb[:, pair * 2:(pair + 1) * 2, :],
                            in_=abp[:, :, 0:NH], func=TT.Copy)
                    tmpn = p5s.tile([128, V, NH], BF16, tag="tmpn")
                    nc.vector.tensor_tensor(out=tmpn[:], in0=vT[:],
                                            in1=absb[:], op=MUL)
                    mxps = mxp.tile([128, NH], F32, space="PSUM", tag="mxps")
                    for vk in range(V):
                        nc.tensor.matmul(out=mxps[:],
                                         lhsT=cb[:, CB_WO:CB_WO + 128],
                                         rhs=tmpn[:, vk, :],
                                         start=(vk == 0), stop=(vk == V - 1))
                    mx = p5s.tile([128, NH], F32, tag="mx")
                    nc.scalar.activation(out=mx[:], in_=mxps[:],
                                         func=TT.Identity,
                                         bias=cf[:, CF_OB:CF_OB + 1])
                    nc.sync.dma_start(out=OUTC[vq], in_=mx[:])

    nc.compile()
    return nc


# ================================================================== kernel()
_CACHE = {}


def kernel(**inputs):
    in_maps = _host_prep(**inputs)
    if "prog" not in _CACHE:
        _CACHE["prog"] = build_program()
    nc = _CACHE["prog"]
    res = run_bass_kernel_spmd(nc, in_maps, core_ids=list(range(8)))
    out = np.zeros((B, V, N, D), np.float32)
    for k in range(8):
        b, half = k // 2, k % 2
        out[b, :, half * NH:(half + 1) * NH, :] = np.asarray(
            res.results[k]["OUTC"]).transpose(0, 2, 1)
    return out


# revision 16
# speedup vs baseline: 1.9284x; 1.0467x over previous
"""Trainium2 Bass kernel for nn_GATv2Layer4View (4-view GATv2 + inter-view MHA).

Sharding: 8 cores = 4 graphs x 2 destination-halves (500 dst nodes each, all
4 views per core).  Host builds per-core dense multiplicity matrices from
edge_index (index prep only); all numerics run on device.

Math (validated vs the reference): the segment softmax collapses because
scores[e] = s_src[src_e] + s_dst[dst_e] and the dst term is segment-constant,
so with w[n,h] = exp(s_src[n,h]) (s_src nonzero only for global ids < 1000,
the reference's raw-id gather quirk):

  out[b,v,d,:] = sum_s C[d,s] * wsel[s,h] * h[b,v,s] /
                 ( indeg(d) + sum_s0 D[d,s0] (w0[s0,h]-1) )

where C[d,s] = multiplicity of in-block edge (s->d) (self-loops included),
D[d,s0] = multiplicity of edges from global src < 1000, and wsel = w0 on the
b=0 cores and 1 elsewhere.  Both contractions run as dense bf16 matmuls
against host-uploaded C^T / D^T; the numerator is produced directly in
[feature, dst] layout (g^T @ C^T) so no transposes are needed anywhere.
The per-node 4-view MHA runs on-chip with PSUM-accumulated aggregation.
"""
import math

import numpy as np
import ml_dtypes

import concourse.bacc as bacc
import concourse.bass as bass
import concourse.mybir as mybir
from concourse.tile import TileContext
from concourse.bass_utils import run_bass_kernel_spmd

# ---------------------------------------------------------------- drain patch
# This container's walrus only accepts one sync-wait on the NO_STRUCT Drain
# encoding; carry each global-clock component on its own single-wait SP nop.
import concourse.tile as _tile_mod
from concourse.vector_clock import ScopedClock, VectorClock


def _patched_drain_and_barrier(self, tick_clock, wait_clock):
    gc = tick_clock.global_clock
    n = len(gc)
    for i in range(n):
        t = gc[i]
        if t > 0:
            v = VectorClock([0] * i + [t] + [0] * (n - 1 - i))
            nop = self.nc.sync.nop(nofuse=True)
            wait_clock.add_sem_waits(nop.ins, ScopedClock({None: v}))
    self.nc.sync.drain()
    self.nc.all_engine_barrier()
    assert self.sems is not None
    popped = self.nc._tile_sem_poison_stack.pop()
    assert popped is self._sem_poison
    self.nc.clear_and_free_semaphores(list(self.sems.allocated().values()))
    self.nc.all_engine_barrier()


_tile_mod.TileContext._drain_and_barrier = _patched_drain_and_barrier
# ----------------------------------------------------------------------------

F32 = mybir.dt.float32
BF16 = mybir.dt.bfloat16
BF = ml_dtypes.bfloat16

B, V, N, IN_F, HEADS, OUT_F = 4, 4, 1000, 64, 4, 32
D = HEADS * OUT_F          # 128
NTOT = B * N               # 4000
NH = 500                   # dst nodes per core (half graph)
NKT = 8                    # src tiles (1000 = 7*128 + 104)
SQ = 1.0 / math.sqrt(32.0)

# CONSTB (bf16) column layout
CB_WT = 0          # [64 rows, 128]  W^T
CB_ATT = 128       # [128, 4]        att_src block-diag
CB_WQ = 132        # [128, 128]      in_proj_w[:D]^T
CB_WK = 260
CB_WV = 388
CB_WO = 516        # out_proj_w^T
CB_ONE = 644       # [128, 32]       per-head ones (j -> head j%4)
CB_BD = 676        # [128, 128]      head->(h,d) expansion, vk-block-replicated
CB_SEL4 = 804      # [128, 4]        (vk-block,h)->h reduction
CB_REP16 = 808     # [4 rows, 128]   h->(vk-block,h) replication
CB_N = 936

# CONSTF (f32) column layout
CF_QB, CF_KB, CF_VB, CF_OB, CF_ISB0, CF_ISB0C = 0, 1, 2, 3, 4, 5
CF_INDEG = 6       # [4 rows, 500]
CF_N = 506


def _pt(kt):
    return 128 if kt < 7 else 104


# ============================================================= host-side prep
def _mult_matrix(dl, sl, rows, cols):
    """Dense multiplicity matrix [rows, cols] from index pairs."""
    idx = dl.astype(np.int64) * cols + sl.astype(np.int64)
    return np.bincount(idx, minlength=rows * cols).reshape(rows, cols)


def _kt_pack(mat_t):
    """[1000, 500] -> [128, 8*500] (src padded to 1024, kt-major)."""
    p = np.zeros((1024, NH), np.float32)
    p[:N] = mat_t
    return np.ascontiguousarray(
        p.reshape(NKT, 128, NH).transpose(1, 0, 2).reshape(128, NKT * NH)
    ).astype(BF)


def _host_prep(x, W, att, in_proj_w, in_proj_b, out_proj_w, out_proj_b,
               bias, edge_index):
    x = np.asarray(x, np.float32)
    ei = np.asarray(edge_index)
    src = np.concatenate([ei[0], np.arange(NTOT)]).astype(np.int64)
    dst = np.concatenate([ei[1], np.arange(NTOT)]).astype(np.int64)
    indeg_full = np.bincount(dst, minlength=NTOT).astype(np.float32)

    constb = np.zeros((128, CB_N), np.float32)
    constb[:IN_F, CB_WT:CB_WT + 128] = np.asarray(W, np.float32).T
    attf = np.asarray(att, np.float32)
    for h in range(HEADS):
        constb[h * 32:(h + 1) * 32, CB_ATT + h] = attf[0, h, :32]
        for j in range(32):
            if j % 4 == h:
                constb[h * 32:(h + 1) * 32, CB_ONE + j] = 1.0
        for vk in range(V):
            constb[vk * 32 + h, CB_BD + h * 32:CB_BD + (h + 1) * 32] = 1.0
            constb[vk * 32 + h, CB_SEL4 + h] = 1.0
            constb[h, CB_REP16 + vk * 32 + h] = 1.0
    ipw = np.asarray(in_proj_w, np.float32)
    constb[:, CB_WQ:CB_WQ + 128] = ipw[:D].T
    constb[:, CB_WK:CB_WK + 128] = ipw[D:2 * D].T
    constb[:, CB_WV:CB_WV + 128] = ipw[2 * D:].T
    constb[:, CB_WO:CB_WO + 128] = np.asarray(out_proj_w, np.float32).T
    constb = constb.astype(BF)

    ipb = np.asarray(in_proj_b, np.float32)
    x00t = np.ascontiguousarray(x[0, 0].T).astype(BF)

    in_maps = []
    for k in range(8):
        b, half = k // 2, k % 2
        base = b * N + half * NH
        m = (dst >= base) & (dst < base + NH)
        dl = dst[m] - base
        sl = src[m]
        in_blk = sl // N == b
        C = _mult_matrix(dl[in_blk], sl[in_blk] - b * N, NH, N)
        g0 = sl < N
        Dm = _mult_matrix(dl[g0], sl[g0], NH, N)

        constf = np.zeros((128, CF_N), np.float32)
        constf[:, CF_QB] = ipb[:D] * SQ
        constf[:, CF_KB] = ipb[D:2 * D]
        constf[:, CF_VB] = ipb[2 * D:]
        constf[:, CF_OB] = (np.asarray(out_proj_b, np.float32)
                            + np.asarray(bias, np.float32))
        constf[:, CF_ISB0] = 1.0 if b == 0 else 0.0
        constf[:, CF_ISB0C] = 0.0 if b == 0 else 1.0
        constf[:4, CF_INDEG:] = indeg_full[base:base + NH][None, :]

        in_maps.append({
            "X00T": x00t,
            "XT": np.ascontiguousarray(
                x[b].transpose(2, 0, 1).reshape(IN_F, V * N)).astype(BF),
            "CT": _kt_pack(C.T),
            "DT": _kt_pack(Dm.T),
            "CONSTB": constb,
            "CONSTF": np.ascontiguousarray(constf),
        })
    return in_maps


# ============================================================ device program
def build_program():
    nc = bacc.Bacc("TRN2", target_bir_lowering=False)

    X00T = nc.dram_tensor("X00T", [IN_F, N], BF16, kind="ExternalInput")
    XT = nc.dram_tensor("XT", [IN_F, V * N], BF16, kind="ExternalInput")
    CTD = nc.dram_tensor("CT", [128, NKT * NH], BF16, kind="ExternalInput")
    DTD = nc.dram_tensor("DT", [128, NKT * NH], BF16, kind="ExternalInput")
    CONSTB = nc.dram_tensor("CONSTB", [128, CB_N], BF16, kind="ExternalInput")
    CONSTF = nc.dram_tensor("CONSTF", [128, CF_N], F32, kind="ExternalInput")
    OUTC = nc.dram_tensor("OUTC", [V, D, NH], F32, kind="ExternalOutput")

    TT = mybir.ActivationFunctionType
    MUL = mybir.AluOpType.mult

    with TileContext(nc) as tc:
        with (
            tc.tile_pool(name="cons", bufs=1) as cons,
            tc.tile_pool(name="per", bufs=1) as per,
        ):
            # uploads: critical-path first
            x00t = cons.tile([IN_F, N], BF16, tag="x00t")
            nc.sync.dma_start(out=x00t[:], in_=X00T[:])
            cb = cons.tile([128, CB_N], BF16, tag="cb")
            nc.scalar.dma_start(out=cb[:], in_=CONSTB[:])
            cf = cons.tile([128, CF_N], F32, tag="cf")
            nc.scalar.dma_start(out=cf[:], in_=CONSTF[:])
            xt = cons.tile([IN_F, V * N], BF16, tag="xt")
            nc.sync.dma_start(out=xt[:], in_=XT[:])
            ct = cons.tile([128, NKT * NH], BF16, tag="ct")
            nc.sync.dma_start(out=ct[:], in_=CTD[:])
            dt = cons.tile([128, NKT * NH], BF16, tag="dt")
            nc.sync.dma_start(out=dt[:], in_=DTD[:])

            wt = cb[0:IN_F, CB_WT:CB_WT + 128]

            lrT = per.tile([128, N], BF16, tag="lrT")
            w_sb = per.tile([128, NKT, HEADS], F32, tag="w_sb")
            wsel = per.tile([128, NKT, HEADS], F32, tag="wsel")
            wm1b = per.tile([128, NKT, HEADS], BF16, tag="wm1b")
            g_sb = per.tile([128, NKT, V * D], BF16, tag="g_sb")
            gatT = per.tile([128, V, NH], BF16, tag="gatT")
            qT = per.tile([128, V, NH], BF16, tag="qT")
            kT = per.tile([128, V, NH], BF16, tag="kT")
            vT = per.tile([128, V, NH], BF16, tag="vT")

            # garbage rows (kt=7, p>104) must still hold benign values
            nc.gpsimd.memset(w_sb[:].bitcast(mybir.dt.int32), 0x3F800000)

            # ---------------- window A: P2 w-tables, P1+P3, den
            CHK = 250
            with (
                tc.tile_pool(name="p2s", bufs=2) as p2s,
                tc.tile_pool(name="p2p", bufs=2, space="PSUM") as p2p,
                tc.tile_pool(name="p2q", bufs=2, space="PSUM") as p2q,
                tc.tile_pool(name="p1p", bufs=3, space="PSUM") as p1p,
                tc.tile_pool(name="p4d", bufs=1, space="PSUM") as p4d,
                tc.tile_pool(name="p4s", bufs=2) as p4s,
            ):
                for c in range(4):
                    sl = slice(c * CHK, (c + 1) * CHK)
                    h0 = p2p.tile([128, CHK], F32, space="PSUM", tag="h0")
                    nc.tensor.matmul(out=h0[:], lhsT=wt, rhs=x00t[:, sl],
                                     start=True, stop=True)
                    sc2 = p2s.tile([128, CHK], BF16, tag="sc2")
                    nc.scalar.activation(out=sc2[:], in_=h0[:],
                                         func=TT.Copy, scale=0.2)
                    nc.vector.tensor_tensor(out=lrT[:, sl], in0=h0[:],
                                            in1=sc2[:],
                                            op=mybir.AluOpType.max)
                isb0c_b = bass.AP(cf.tensor, cf[:, CF_ISB0C:CF_ISB0C + 1].offset,
                                  [cf[:, 0:1].ap[0], [0, 4 * HEADS]])
                for hf in range(2):
                    for nt in range(hf * 4, hf * 4 + 4):
                        p = _pt(nt)
                        sc = p2q.tile([128, HEADS], F32, space="PSUM", tag="sc")
                        nc.tensor.matmul(out=sc[:p, :],
                                         lhsT=lrT[:, nt * 128:nt * 128 + p],
                                         rhs=cb[:, CB_ATT:CB_ATT + 4],
                                         start=True, stop=True)
                        nc.scalar.activation(out=w_sb[:p, nt, :], in_=sc[:p, :],
                                             func=TT.Exp)
                    wv_ = w_sb[:, hf * 4:(hf + 1) * 4, :].rearrange(
                        "p a b -> p (a b)")
                    nc.vector.tensor_scalar_add(
                        out=wm1b[:, hf * 4:(hf + 1) * 4, :].rearrange(
                            "p a b -> p (a b)"), in0=wv_, scalar1=-1.0)
                    # wsel = w*isb0 + (1-isb0)
                    nc.vector.scalar_tensor_tensor(
                        out=wsel[:, hf * 4:(hf + 1) * 4, :].rearrange(
                            "p a b -> p (a b)"), in0=wv_,
                        scalar=cf[:, CF_ISB0:CF_ISB0 + 1], in1=isb0c_b,
                        op0=MUL, op1=mybir.AluOpType.add)

                # P1+P3 per src tile: h = x@W^T, g = wsel*h
                for kt in range(NKT):
                    p = _pt(kt)
                    hps = p1p.tile([128, V * D], F32, space="PSUM", tag="hps")
                    for v in range(V):
                        nc.tensor.matmul(
                            out=hps[:p, v * D:(v + 1) * D],
                            lhsT=xt[:, v * N + kt * 128:v * N + kt * 128 + p],
                            rhs=wt, start=True, stop=True)
                    wb = bass.AP(wsel.tensor, wsel[:p, kt, :].offset,
                                 [wsel[:p, kt, :].ap[0], [0, V], [1, HEADS],
                                  [0, OUT_F]])
                    nc.vector.tensor_tensor(
                        out=g_sb[:p, kt, :].rearrange(
                            "p (a b c) -> p a b c", a=V, b=HEADS),
                        in0=hps[:p, :].rearrange(
                            "p (a b c) -> p a b c", a=V, b=HEADS),
                        in1=wb, op=MUL)

                # denT = wm1^T @ D^T (+indeg) -> 1/den
                dens = p4d.tile([HEADS, NH], F32, space="PSUM", tag="dens")
                for kt in range(NKT):
                    p = _pt(kt)
                    nc.tensor.matmul(
                        out=dens[:], lhsT=wm1b[:p, kt, :],
                        rhs=dt[:p, kt * NH:(kt + 1) * NH],
                        start=kt == 0, stop=kt == NKT - 1)
                denf = p4s.tile([HEADS, NH], F32, tag="denf")
                nc.vector.tensor_add(out=denf[:], in0=dens[:],
                                     in1=cf[0:4, CF_INDEG:])
                rec4g = p4s.tile([HEADS, NH], BF16, tag="rec4g")
                with nc.allow_low_precision(reason="bf16 alpha ok"):
                    nc.vector.reciprocal(out=rec4g[:], in_=denf[:])

            # ---------------- window B: numT_v = g_v^T @ C^T, gatT, q/k/v
            with (
                tc.tile_pool(name="p4p", bufs=1, space="PSUM") as p4p,
                tc.tile_pool(name="p4q", bufs=1, space="PSUM") as p4q,
                tc.tile_pool(name="p4t", bufs=2) as p4t,
                tc.tile_pool(name="p5p", bufs=3, space="PSUM") as p5p,
            ):
                recx = p4q.tile([128, NH], F32, space="PSUM", tag="recx")
                nc.tensor.matmul(out=recx[:], lhsT=cb[0:4, CB_BD:CB_BD + 128],
                                 rhs=rec4g[:], start=True, stop=True)
                recxb = p4t.tile([128, NH], BF16, tag="recxb")
                nc.scalar.activation(out=recxb[:], in_=recx[:], func=TT.Copy)
                nums = p4p.tile([128, V, 512], F32, space="PSUM", tag="nums")
                for v in range(V):
                    for kt in range(NKT):
                        p = _pt(kt)
                        nc.tensor.matmul(
                            out=nums[:, v, 0:NH],
                            lhsT=g_sb[:p, kt, v * D:(v + 1) * D],
                            rhs=ct[:p, kt * NH:(kt + 1) * NH],
                            start=kt == 0, stop=kt == NKT - 1)
                    nc.vector.tensor_tensor(out=gatT[:, v, :],
                                            in0=nums[:, v, 0:NH],
                                            in1=recxb[:], op=MUL)
                    for (col, dstt, bcol, scale, eng) in (
                        (CB_WQ, qT, CF_QB, SQ, "q"),
                        (CB_WK, kT, CF_KB, 1.0, "k"),
                        (CB_WV, vT, CF_VB, 1.0, "v"),
                    ):
                        pp = p5p.tile([128, NH], F32, space="PSUM", tag="qkv")
                        nc.tensor.matmul(out=pp[:],
                                         lhsT=cb[:, col:col + 128],
                                         rhs=gatT[:, v, :],
                                         start=True, stop=True)
                        if eng == "k":
                            nc.vector.tensor_scalar_add(
                                out=dstt[:, v, :], in0=pp[:],
                                scalar1=cf[:, bcol:bcol + 1])
                        else:
                            nc.scalar.activation(
                                out=dstt[:, v, :], in_=pp[:], func=TT.Identity,
                                bias=cf[:, bcol:bcol + 1], scale=scale)

            # ---------------- P5b: attention over views + out proj
            with (
                tc.tile_pool(name="p5s", bufs=2) as p5s,
                tc.tile_pool(name="p5r", bufs=4) as p5r,
                tc.tile_pool(name="p5e", bufs=2) as p5e,
                tc.tile_pool(name="a16", bufs=1, space="PSUM") as a16,
                tc.tile_pool(name="apb", bufs=1, space="PSUM") as apb,
                tc.tile_pool(name="mxp", bufs=2, space="PSUM") as mxp,
            ):
                for vq in range(V):
                    prod = p5r.tile([128, V, NH], BF16, tag="prod")
                    qv = qT[:, vq, :]
                    qb_ap = bass.AP(qT.tensor, qv.offset,
                                    [qv.ap[0], [0, V], [1, NH]])
                    nc.vector.tensor_tensor(out=prod[:], in0=qb_ap,
                                            in1=kT[:], op=MUL)
                    # vk blocks at partition 0/32 of two half-tiles (PE can
                    # only address partition bases 0/32/64)
                    s2 = a16.tile([64, 2, 512], F32, space="PSUM", tag="s2")
                    for vk in range(V):
                        nc.tensor.matmul(
                            out=s2[(vk % 2) * 32:(vk % 2) * 32 + 32, vk // 2, 0:NH],
                            lhsT=cb[:, CB_ONE:CB_ONE + 32],
                            rhs=prod[:, vk, :], start=True, stop=True)
                    e2 = p5e.tile([64, 2, NH], BF16, tag="e2")
                    nc.scalar.activation(out=e2[:], in_=s2[:, :, 0:NH], func=TT.Exp)
                    ssum = a16.tile([HEADS, NH], F32, space="PSUM", tag="ssum")
                    for half in range(2):
                        nc.tensor.matmul(out=ssum[:],
                                         lhsT=cb[0:64, CB_SEL4:CB_SEL4 + 4],
                                         rhs=e2[:, half, :],
                                         start=(half == 0), stop=(half == 1))
                    rec4 = p5s.tile([HEADS, NH], BF16, tag="rec4")
                    with nc.allow_low_precision(reason="bf16 softmax recip"):
                        nc.vector.reciprocal(out=rec4[:], in_=ssum[:])
                    rec64 = a16.tile([64, NH], F32, space="PSUM", tag="r64")
                    nc.tensor.matmul(out=rec64[:],
                                     lhsT=cb[0:4, CB_REP16:CB_REP16 + 64],
                                     rhs=rec4[:], start=True, stop=True)
                    rec64b = p5e.tile([64, NH], BF16, tag="r64b")
                    nc.scalar.activation(out=rec64b[:], in_=rec64[:],
                                         func=TT.Copy)
                    en2 = p5e.tile([64, 2, NH], BF16, tag="en2")
                    r64v = rec64b[:]
                    r64_ap = bass.AP(rec64b.tensor, r64v.offset,
                                     [r64v.ap[0], [0, 2], [1, NH]])
                    nc.vector.tensor_tensor(out=en2[:], in0=e2[:],
                                            in1=r64_ap, op=MUL)
                    absb = p5s.tile([128, V, NH], BF16, tag="absb")
                    for pair in range(2):
                        abp = apb.tile([128, 2, 512], F32, space="PSUM",
                                       tag="abp")
                        for j in range(2):
                            vk = pair * 2 + j
                            blk = (vk % 2) * 32
                            nc.tensor.matmul(
                                out=abp[:, j, 0:NH],
                                lhsT=cb[blk:blk + 4, CB_BD:CB_BD + 128],
                                rhs=en2[blk:blk + 4, vk // 2, :],
                                start=True, stop=True)
                        nc.scalar.activation(
                            out=absb[:, pair * 2:(pair + 1) * 2, :],
                            in_=abp[:, :, 0:NH], func=TT.Copy)
                    tmpn = p5s.tile([128, V, NH], BF16, tag="tmpn")
                    nc.vector.tensor_tensor(out=tmpn[:], in0=vT[:],
                                            in1=absb[:], op=MUL)
                    mxps = mxp.tile([128, NH], F32, space="PSUM", tag="mxps")
                    for vk in range(V):
                        nc.tensor.matmul(out=mxps[:],
                                         lhsT=cb[:, CB_WO:CB_WO + 128],
                                         rhs=tmpn[:, vk, :],
                                         start=(vk == 0), stop=(vk == V - 1))
                    mx = p5s.tile([128, NH], F32, tag="mx")
                    nc.scalar.activation(out=mx[:], in_=mxps[:],
                                         func=TT.Identity,
                                         bias=cf[:, CF_OB:CF_OB + 1])
                    nc.sync.dma_start(out=OUTC[vq], in_=mx[:])

    nc.compile()
    return nc


# ================================================================== kernel()
_CACHE = {}


def kernel(**inputs):
    in_maps = _host_prep(**inputs)
    if "prog" not in _CACHE:
        _CACHE["prog"] = build_program()
    nc = _CACHE["prog"]
    res = run_bass_kernel_spmd(nc, in_maps, core_ids=list(range(8)))
    out = np.zeros((B, V, N, D), np.float32)
    for k in range(8):
        b, half = k // 2, k % 2
        out[b, :, half * NH:(half + 1) * NH, :] = np.asarray(
            res.results[k]["OUTC"]).transpose(0, 2, 1)
    return out


# revision 18
# speedup vs baseline: 1.9789x; 1.0262x over previous
"""Trainium2 Bass kernel for nn_GATv2Layer4View (4-view GATv2 + inter-view MHA).

Sharding: 8 cores = 4 graphs x 2 destination-halves (500 dst nodes each, all
4 views per core).  Host builds per-core dense multiplicity matrices from
edge_index (index prep only); all numerics run on device.

Math (validated vs the reference): the segment softmax collapses because
scores[e] = s_src[src_e] + s_dst[dst_e] and the dst term is segment-constant,
so with w[n,h] = exp(s_src[n,h]) (s_src nonzero only for global ids < 1000,
the reference's raw-id gather quirk):

  out[b,v,d,:] = sum_s C[d,s] * wsel[s,h] * h[b,v,s] /
                 ( indeg(d) + sum_s0 D[d,s0] (w0[s0,h]-1) )

where C[d,s] = multiplicity of in-block edge (s->d) (self-loops included),
D[d,s0] = multiplicity of edges from global src < 1000, and wsel = w0 on the
b=0 cores and 1 elsewhere.  Both contractions run as dense bf16 matmuls
against host-uploaded C^T / D^T; the numerator is produced directly in
[feature, dst] layout (g^T @ C^T) so no transposes are needed anywhere.
The per-node 4-view MHA runs on-chip with PSUM-accumulated aggregation.
"""
import math

import numpy as np
import ml_dtypes

import concourse.bacc as bacc
import concourse.bass as bass
import concourse.mybir as mybir
from concourse.tile import TileContext
from concourse.bass_utils import run_bass_kernel_spmd

# ---------------------------------------------------------------- drain patch
# This container's walrus only accepts one sync-wait on the NO_STRUCT Drain
# encoding; carry each global-clock component on its own single-wait SP nop.
import concourse.tile as _tile_mod
from concourse.vector_clock import ScopedClock, VectorClock


def _patched_drain_and_barrier(self, tick_clock, wait_clock):
    gc = tick_clock.global_clock
    n = len(gc)
    for i in range(n):
        t = gc[i]
        if t > 0:
            v = VectorClock([0] * i + [t] + [0] * (n - 1 - i))
            nop = self.nc.sync.nop(nofuse=True)
            wait_clock.add_sem_waits(nop.ins, ScopedClock({None: v}))
    self.nc.sync.drain()
    self.nc.all_engine_barrier()
    assert self.sems is not None
    popped = self.nc._tile_sem_poison_stack.pop()
    assert popped is self._sem_poison
    self.nc.clear_and_free_semaphores(list(self.sems.allocated().values()))
    self.nc.all_engine_barrier()


_tile_mod.TileContext._drain_and_barrier = _patched_drain_and_barrier
# ----------------------------------------------------------------------------

F32 = mybir.dt.float32
BF16 = mybir.dt.bfloat16
BF = ml_dtypes.bfloat16

B, V, N, IN_F, HEADS, OUT_F = 4, 4, 1000, 64, 4, 32
D = HEADS * OUT_F          # 128
NTOT = B * N               # 4000
NH = 500                   # dst nodes per core (half graph)
NKT = 8                    # src tiles (1000 = 7*128 + 104)
SQ = 1.0 / math.sqrt(32.0)

# CONSTB (bf16) column layout
CB_WT = 0          # [64 rows, 128]  W^T
CB_ATT = 128       # [128, 4]        att_src block-diag
CB_WQ = 132        # [128, 128]      in_proj_w[:D]^T
CB_WK = 260
CB_WV = 388
CB_WO = 516        # out_proj_w^T
CB_ONE = 644       # [128, 32]       per-head ones (j -> head j%4)
CB_BD = 676        # [128, 128]      head->(h,d) expansion, vk-block-replicated
CB_SEL4 = 804      # [128, 4]        (vk-block,h)->h reduction
CB_REP16 = 808     # [4 rows, 128]   h->(vk-block,h) replication
CB_N = 936

# CONSTF (f32) column layout
CF_QB, CF_KB, CF_VB, CF_OB, CF_ISB0, CF_ISB0C = 0, 1, 2, 3, 4, 5
CF_INDEG = 6       # [4 rows, 500]
CF_N = 506


def _pt(kt):
    return 128 if kt < 7 else 104


# ============================================================= host-side prep
def _mult_matrix(dl, sl, rows, cols):
    """Dense multiplicity matrix [rows, cols] from index pairs."""
    idx = dl.astype(np.int64) * cols + sl.astype(np.int64)
    return np.bincount(idx, minlength=rows * cols).reshape(rows, cols)


def _kt_pack(mat_t):
    """[1000, 500] -> [128, 8*500] (src padded to 1024, kt-major)."""
    p = np.zeros((1024, NH), np.float32)
    p[:N] = mat_t
    return np.ascontiguousarray(
        p.reshape(NKT, 128, NH).transpose(1, 0, 2).reshape(128, NKT * NH)
    ).astype(BF)


def _host_prep(x, W, att, in_proj_w, in_proj_b, out_proj_w, out_proj_b,
               bias, edge_index):
    x = np.asarray(x, np.float32)
    ei = np.asarray(edge_index)
    src = np.concatenate([ei[0], np.arange(NTOT)]).astype(np.int64)
    dst = np.concatenate([ei[1], np.arange(NTOT)]).astype(np.int64)
    indeg_full = np.bincount(dst, minlength=NTOT).astype(np.float32)

    constb = np.zeros((128, CB_N), np.float32)
    constb[:IN_F, CB_WT:CB_WT + 128] = np.asarray(W, np.float32).T
    attf = np.asarray(att, np.float32)
    for h in range(HEADS):
        constb[h * 32:(h + 1) * 32, CB_ATT + h] = attf[0, h, :32]
        for j in range(32):
            if j % 4 == h:
                constb[h * 32:(h + 1) * 32, CB_ONE + j] = 1.0
        for vk in range(V):
            constb[vk * 32 + h, CB_BD + h * 32:CB_BD + (h + 1) * 32] = 1.0
            constb[vk * 32 + h, CB_SEL4 + h] = 1.0
            constb[h, CB_REP16 + vk * 32 + h] = 1.0
    ipw = np.asarray(in_proj_w, np.float32)
    constb[:, CB_WQ:CB_WQ + 128] = ipw[:D].T
    constb[:, CB_WK:CB_WK + 128] = ipw[D:2 * D].T
    constb[:, CB_WV:CB_WV + 128] = ipw[2 * D:].T
    constb[:, CB_WO:CB_WO + 128] = np.asarray(out_proj_w, np.float32).T
    constb = constb.astype(BF)

    ipb = np.asarray(in_proj_b, np.float32)
    x00t = np.ascontiguousarray(x[0, 0].T).astype(BF)

    in_maps = []
    for k in range(8):
        b, half = k // 2, k % 2
        base = b * N + half * NH
        m = (dst >= base) & (dst < base + NH)
        dl = dst[m] - base
        sl = src[m]
        in_blk = sl // N == b
        C = _mult_matrix(dl[in_blk], sl[in_blk] - b * N, NH, N)
        g0 = sl < N
        Dm = _mult_matrix(dl[g0], sl[g0], NH, N)

        constf = np.zeros((128, CF_N), np.float32)
        constf[:, CF_QB] = ipb[:D] * SQ
        constf[:, CF_KB] = ipb[D:2 * D]
        constf[:, CF_VB] = ipb[2 * D:]
        constf[:, CF_OB] = (np.asarray(out_proj_b, np.float32)
                            + np.asarray(bias, np.float32))
        constf[:, CF_ISB0] = 1.0 if b == 0 else 0.0
        constf[:, CF_ISB0C] = 0.0 if b == 0 else 1.0
        constf[:4, CF_INDEG:] = indeg_full[base:base + NH][None, :]

        in_maps.append({
            "X00T": x00t,
            "XT": np.ascontiguousarray(
                x[b].transpose(2, 0, 1).reshape(IN_F, V * N)).astype(BF),
            "CT": _kt_pack(C.T),
            "DT": _kt_pack(Dm.T),
            "CONSTB": constb,
            "CONSTF": np.ascontiguousarray(constf),
        })
    return in_maps


# ============================================================ device program
def build_program():
    nc = bacc.Bacc("TRN2", target_bir_lowering=False)

    X00T = nc.dram_tensor("X00T", [IN_F, N], BF16, kind="ExternalInput")
    XT = nc.dram_tensor("XT", [IN_F, V * N], BF16, kind="ExternalInput")
    CTD = nc.dram_tensor("CT", [128, NKT * NH], BF16, kind="ExternalInput")
    DTD = nc.dram_tensor("DT", [128, NKT * NH], BF16, kind="ExternalInput")
    CONSTB = nc.dram_tensor("CONSTB", [128, CB_N], BF16, kind="ExternalInput")
    CONSTF = nc.dram_tensor("CONSTF", [128, CF_N], F32, kind="ExternalInput")
    OUTC = nc.dram_tensor("OUTC", [V, D, NH], F32, kind="ExternalOutput")

    TT = mybir.ActivationFunctionType
    MUL = mybir.AluOpType.mult

    with TileContext(nc) as tc:
        with (
            tc.tile_pool(name="cons", bufs=1) as cons,
            tc.tile_pool(name="per", bufs=1) as per,
        ):
            # uploads: critical-path first
            cb = cons.tile([128, CB_N], BF16, tag="cb")
            nc.scalar.dma_start(out=cb[:], in_=CONSTB[:])
            x00t = cons.tile([IN_F, N], BF16, tag="x00t")
            nc.sync.dma_start(out=x00t[:], in_=X00T[:])
            cf = cons.tile([128, CF_N], F32, tag="cf")
            nc.scalar.dma_start(out=cf[:], in_=CONSTF[:])
            dt = cons.tile([128, NKT * NH], BF16, tag="dt")
            nc.sync.dma_start(out=dt[:], in_=DTD[:])
            xt = cons.tile([IN_F, V * N], BF16, tag="xt")
            nc.sync.dma_start(out=xt[:], in_=XT[:])
            ct = cons.tile([128, NKT * NH], BF16, tag="ct")
            nc.sync.dma_start(out=ct[:], in_=CTD[:])

            wt = cb[0:IN_F, CB_WT:CB_WT + 128]

            lrT = per.tile([128, N], BF16, tag="lrT")
            w_sb = per.tile([128, NKT, HEADS], F32, tag="w_sb")
            wsel = per.tile([128, NKT, HEADS], F32, tag="wsel")
            wm1b = per.tile([128, NKT, HEADS], BF16, tag="wm1b")
            g_sb = per.tile([128, NKT, V * D], BF16, tag="g_sb")
            gatT = per.tile([128, V, NH], BF16, tag="gatT")
            qT = per.tile([128, V, NH], BF16, tag="qT")
            kT = per.tile([128, V, NH], BF16, tag="kT")
            vT = per.tile([128, V, NH], BF16, tag="vT")

            # garbage rows (kt=7, p>104) must still hold benign values
            nc.gpsimd.memset(w_sb[:].bitcast(mybir.dt.int32), 0x3F800000)

            # ---------------- window A: P2 w-tables, P1+P3, den
            CHK = 250
            with (
                tc.tile_pool(name="p2s", bufs=2) as p2s,
                tc.tile_pool(name="p2p", bufs=2, space="PSUM") as p2p,
                tc.tile_pool(name="p2q", bufs=2, space="PSUM") as p2q,
                tc.tile_pool(name="p1p", bufs=3, space="PSUM") as p1p,
                tc.tile_pool(name="p4d", bufs=1, space="PSUM") as p4d,
                tc.tile_pool(name="p4s", bufs=2) as p4s,
            ):
                for c in range(4):
                    sl = slice(c * CHK, (c + 1) * CHK)
                    h0 = p2p.tile([128, CHK], F32, space="PSUM", tag="h0")
                    nc.tensor.matmul(out=h0[:], lhsT=wt, rhs=x00t[:, sl],
                                     start=True, stop=True)
                    sc2 = p2s.tile([128, CHK], BF16, tag="sc2")
                    nc.scalar.activation(out=sc2[:], in_=h0[:],
                                         func=TT.Copy, scale=0.2)
                    nc.vector.tensor_tensor(out=lrT[:, sl], in0=h0[:],
                                            in1=sc2[:],
                                            op=mybir.AluOpType.max)
                isb0c_b = bass.AP(cf.tensor, cf[:, CF_ISB0C:CF_ISB0C + 1].offset,
                                  [cf[:, 0:1].ap[0], [0, 4 * HEADS]])
                for hf in range(2):
                    for nt in range(hf * 4, hf * 4 + 4):
                        p = _pt(nt)
                        sc = p2q.tile([128, HEADS], F32, space="PSUM", tag="sc")
                        nc.tensor.matmul(out=sc[:p, :],
                                         lhsT=lrT[:, nt * 128:nt * 128 + p],
                                         rhs=cb[:, CB_ATT:CB_ATT + 4],
                                         start=True, stop=True)
                        nc.scalar.activation(out=w_sb[:p, nt, :], in_=sc[:p, :],
                                             func=TT.Exp)
                    wv_ = w_sb[:, hf * 4:(hf + 1) * 4, :].rearrange(
                        "p a b -> p (a b)")
                    nc.vector.tensor_scalar_add(
                        out=wm1b[:, hf * 4:(hf + 1) * 4, :].rearrange(
                            "p a b -> p (a b)"), in0=wv_, scalar1=-1.0)
                    # wsel = w*isb0 + (1-isb0)
                    nc.vector.scalar_tensor_tensor(
                        out=wsel[:, hf * 4:(hf + 1) * 4, :].rearrange(
                            "p a b -> p (a b)"), in0=wv_,
                        scalar=cf[:, CF_ISB0:CF_ISB0 + 1], in1=isb0c_b,
                        op0=MUL, op1=mybir.AluOpType.add)

                # denT = wm1^T @ D^T first (short PE ops, unblocks rec chain)
                dens = p4d.tile([HEADS, NH], F32, space="PSUM", tag="dens")
                for kt in range(NKT):
                    p = _pt(kt)
                    nc.tensor.matmul(
                        out=dens[:], lhsT=wm1b[:p, kt, :],
                        rhs=dt[:p, kt * NH:(kt + 1) * NH],
                        start=kt == 0, stop=kt == NKT - 1)
                denf = p4s.tile([HEADS, NH], F32, tag="denf")
                nc.vector.tensor_add(out=denf[:], in0=dens[:],
                                     in1=cf[0:4, CF_INDEG:])
                rec4g = p4s.tile([HEADS, NH], BF16, tag="rec4g")
                with nc.allow_low_precision(reason="bf16 alpha ok"):
                    nc.vector.reciprocal(out=rec4g[:], in_=denf[:])

                # P1+P3 per src tile: h = x@W^T, g = wsel*h
                for kt in range(NKT):
                    p = _pt(kt)
                    hps = p1p.tile([128, V * D], F32, space="PSUM", tag="hps")
                    for v in range(V):
                        nc.tensor.matmul(
                            out=hps[:p, v * D:(v + 1) * D],
                            lhsT=xt[:, v * N + kt * 128:v * N + kt * 128 + p],
                            rhs=wt, start=True, stop=True)
                    wb = bass.AP(wsel.tensor, wsel[:p, kt, :].offset,
                                 [wsel[:p, kt, :].ap[0], [0, V], [1, HEADS],
                                  [0, OUT_F]])
                    nc.vector.tensor_tensor(
                        out=g_sb[:p, kt, :].rearrange(
                            "p (a b c) -> p a b c", a=V, b=HEADS),
                        in0=hps[:p, :].rearrange(
                            "p (a b c) -> p a b c", a=V, b=HEADS),
                        in1=wb, op=MUL)

            # ---------------- window B: numT_v = g_v^T @ C^T, gatT, q/k/v
            with (
                tc.tile_pool(name="p4p", bufs=1, space="PSUM") as p4p,
                tc.tile_pool(name="p4q", bufs=1, space="PSUM") as p4q,
                tc.tile_pool(name="p4t", bufs=2) as p4t,
                tc.tile_pool(name="p5p", bufs=3, space="PSUM") as p5p,
            ):
                recx = p4q.tile([128, NH], F32, space="PSUM", tag="recx")
                nc.tensor.matmul(out=recx[:], lhsT=cb[0:4, CB_BD:CB_BD + 128],
                                 rhs=rec4g[:], start=True, stop=True)
                recxb = p4t.tile([128, NH], BF16, tag="recxb")
                nc.scalar.activation(out=recxb[:], in_=recx[:], func=TT.Copy)
                nums = p4p.tile([128, V, 512], F32, space="PSUM", tag="nums")
                for v in range(V):
                    for kt in range(NKT):
                        p = _pt(kt)
                        nc.tensor.matmul(
                            out=nums[:, v, 0:NH],
                            lhsT=g_sb[:p, kt, v * D:(v + 1) * D],
                            rhs=ct[:p, kt * NH:(kt + 1) * NH],
                            start=kt == 0, stop=kt == NKT - 1)
                    nc.vector.tensor_tensor(out=gatT[:, v, :],
                                            in0=nums[:, v, 0:NH],
                                            in1=recxb[:], op=MUL)
                    for (col, dstt, bcol, scale, eng) in (
                        (CB_WQ, qT, CF_QB, SQ, "q"),
                        (CB_WK, kT, CF_KB, 1.0, "k"),
                        (CB_WV, vT, CF_VB, 1.0, "v"),
                    ):
                        pp = p5p.tile([128, NH], F32, space="PSUM", tag="qkv")
                        nc.tensor.matmul(out=pp[:],
                                         lhsT=cb[:, col:col + 128],
                                         rhs=gatT[:, v, :],
                                         start=True, stop=True)
                        if eng == "k":
                            nc.vector.tensor_scalar_add(
                                out=dstt[:, v, :], in0=pp[:],
                                scalar1=cf[:, bcol:bcol + 1])
                        else:
                            nc.scalar.activation(
                                out=dstt[:, v, :], in_=pp[:], func=TT.Identity,
                                bias=cf[:, bcol:bcol + 1], scale=scale)

            # ---------------- P5b: attention over views + out proj
            with (
                tc.tile_pool(name="p5s", bufs=2) as p5s,
                tc.tile_pool(name="p5r", bufs=4) as p5r,
                tc.tile_pool(name="p5e", bufs=2) as p5e,
                tc.tile_pool(name="a16", bufs=2, space="PSUM") as a16,
                tc.tile_pool(name="a4", bufs=1, space="PSUM") as a4,
                tc.tile_pool(name="apb", bufs=1, space="PSUM") as apb,
                tc.tile_pool(name="mxp", bufs=2, space="PSUM") as mxp,
            ):
                for vq in range(V):
                    prod = p5r.tile([128, V, NH], BF16, tag="prod")
                    qv = qT[:, vq, :]
                    qb_ap = bass.AP(qT.tensor, qv.offset,
                                    [qv.ap[0], [0, V], [1, NH]])
                    nc.vector.tensor_tensor(out=prod[:], in0=qb_ap,
                                            in1=kT[:], op=MUL)
                    # scores for all (vk,h): vk blocks at partitions vk*32
                    s16 = a16.tile([128, 512], F32, space="PSUM", tag="s16")
                    for vk in range(V):
                        nc.tensor.matmul(
                            out=s16[vk * 32:(vk + 1) * 32, 0:NH],
                            lhsT=cb[:, CB_ONE:CB_ONE + 32],
                            rhs=prod[:, vk, :], start=True, stop=True,
                            tile_position=(0, vk * 32))
                    e16 = p5e.tile([128, NH], BF16, tag="e16")
                    nc.scalar.activation(out=e16[:], in_=s16[:, 0:NH],
                                         func=TT.Exp)
                    ssum = a4.tile([HEADS, NH], F32, space="PSUM", tag="ssum")
                    nc.tensor.matmul(out=ssum[:],
                                     lhsT=cb[:, CB_SEL4:CB_SEL4 + 4],
                                     rhs=e16[:], start=True, stop=True)
                    rec4 = p5s.tile([HEADS, NH], BF16, tag="rec4")
                    with nc.allow_low_precision(reason="bf16 softmax recip"):
                        nc.vector.reciprocal(out=rec4[:], in_=ssum[:])
                    rec16 = a4.tile([128, 512], F32, space="PSUM", tag="r16")
                    nc.tensor.matmul(out=rec16[:, 0:NH],
                                     lhsT=cb[0:4, CB_REP16:CB_REP16 + 128],
                                     rhs=rec4[:], start=True, stop=True)
                    en16 = p5e.tile([128, NH], BF16, tag="en16")
                    nc.vector.tensor_tensor(out=en16[:], in0=e16[:],
                                            in1=rec16[:, 0:NH], op=MUL)
                    absb = p5s.tile([128, V, NH], BF16, tag="absb")
                    for pair in range(2):
                        abp = apb.tile([128, 2, 512], F32, space="PSUM",
                                       tag="abp")
                        for j in range(2):
                            vk = pair * 2 + j
                            blk = vk * 32
                            nc.tensor.matmul(
                                out=abp[:, j, 0:NH],
                                lhsT=cb[blk:blk + 4, CB_BD:CB_BD + 128],
                                rhs=en16[blk:blk + 4, :],
                                start=True, stop=True,
                                tile_position=(blk, 0))
                        nc.scalar.activation(
                            out=absb[:, pair * 2:(pair + 1) * 2, :],
                            in_=abp[:, :, 0:NH], func=TT.Copy)
                    tmpn = p5s.tile([128, V, NH], BF16, tag="tmpn")
                    nc.vector.tensor_tensor(out=tmpn[:], in0=vT[:],
                                            in1=absb[:], op=MUL)
                    mxps = mxp.tile([128, NH], F32, space="PSUM", tag="mxps")
                    for vk in range(V):
                        nc.tensor.matmul(out=mxps[:],
                                         lhsT=cb[:, CB_WO:CB_WO + 128],
                                         rhs=tmpn[:, vk, :],
                                         start=(vk == 0), stop=(vk == V - 1))
                    mx = p5s.tile([128, NH], F32, tag="mx")
                    nc.scalar.activation(out=mx[:], in_=mxps[:],
                                         func=TT.Identity,
                                         bias=cf[:, CF_OB:CF_OB + 1])
                    nc.sync.dma_start(out=OUTC[vq], in_=mx[:])

    nc.compile()
    return nc


# ================================================================== kernel()
_CACHE = {}


def kernel(**inputs):
    in_maps = _host_prep(**inputs)
    if "prog" not in _CACHE:
        _CACHE["prog"] = build_program()
    nc = _CACHE["prog"]
    res = run_bass_kernel_spmd(nc, in_maps, core_ids=list(range(8)))
    out = np.zeros((B, V, N, D), np.float32)
    for k in range(8):
        b, half = k // 2, k % 2
        out[b, :, half * NH:(half + 1) * NH, :] = np.asarray(
            res.results[k]["OUTC"]).transpose(0, 2, 1)
    return out
